# revision 1
# baseline (speedup 1.0000x reference)
"""Trainium2 Bass kernel for causal GQA multi-head attention (nn_MHA_86835648791002).

Sharding: 8 cores = 2 (batch) x 4 (KV-head groups).  Each core computes 4 Q
heads / 1 KV head for one batch element: QKV projection + RoPE, causal
attention (full attn-probs output), and a partial output projection (its head
group's slice of Wo); the host sums the 4 partial out-projections per batch.

All matmuls run in bf16 (PE full rate); softmax statistics in fp32.
RoPE pairs are de-interleaved on the host via a column permutation of
Wq/Wk/bq/bk (dot products over head_dim are permutation invariant).
"""

import sys

sys.path.insert(0, '/opt/trn_rl_repo')
sys.path.insert(0, '/opt/trn_rl_repo/concourse')

import numpy as np
import ml_dtypes

import concourse.bass as bass
import concourse.bacc as bacc
import concourse.mybir as mybir
from concourse.tile import TileContext
from concourse.bass_utils import run_bass_kernel_spmd
from concourse.masks import make_identity

BF = mybir.dt.bfloat16
F32 = mybir.dt.float32
AF = mybir.ActivationFunctionType

B, S, D = 2, 2048, 2048
HQ, HKV, HD = 16, 4, 128
GH = HQ // HKV          # q heads per core = 4
SCALE = 1.0 / float(np.sqrt(D))
NT = S // 128           # 16 seq tiles
NC = S // 512           # 4 seq chunks

_CACHED_NC = None


def build_nc():
    nc = bacc.Bacc(target_bir_lowering=False)

    xT = nc.dram_tensor("xT", [D, S], BF, kind="ExternalInput")
    wq = nc.dram_tensor("wq", [D, GH * HD], BF, kind="ExternalInput")
    wk = nc.dram_tensor("wk", [D, HD], BF, kind="ExternalInput")
    wv = nc.dram_tensor("wv", [D, HD], BF, kind="ExternalInput")
    wo = nc.dram_tensor("wo", [GH * HD, D], BF, kind="ExternalInput")
    bq = nc.dram_tensor("bq", [1, GH * HD], BF, kind="ExternalInput")
    bk = nc.dram_tensor("bk", [1, HD], BF, kind="ExternalInput")
    bv = nc.dram_tensor("bv", [1, HD], BF, kind="ExternalInput")
    cosq = nc.dram_tensor("cosq", [S, 256], F32, kind="ExternalInput")
    sinq = nc.dram_tensor("sinq", [S, 256], F32, kind="ExternalInput")
    cosk = nc.dram_tensor("cosk", [S, 64], F32, kind="ExternalInput")
    sink = nc.dram_tensor("sink", [S, 64], F32, kind="ExternalInput")

    attn_o = [nc.dram_tensor(f"attn{h}", [S, S], F32, kind="ExternalOutput")
              for h in range(GH)]
    outp = nc.dram_tensor("outp", [S, D], F32, kind="ExternalOutput")

    with TileContext(nc) as tc:
        with (
            tc.tile_pool(name="const", bufs=1) as cp,
            tc.tile_pool(name="qkv", bufs=1) as qp,
            tc.tile_pool(name="wop", bufs=1) as wop,
        ):
            ident = cp.tile([128, 128], F32)
            make_identity(nc, ident)
            ones_bf = cp.tile([128, 1], BF)
            nc.vector.memset(ones_bf, 1.0)
            ones1_bf = cp.tile([1, 128], BF)
            nc.vector.memset(ones1_bf, 1.0)
            ones_row_f = cp.tile([1, 128], F32)
            nc.vector.memset(ones_row_f, 1.0)
            # additive causal mask for [sq, sk] diag tiles: 0 if sk<=sq else -1e6
            mask_add = cp.tile([128, 128], F32)
            nc.gpsimd.memset(mask_add, 0.0)
            nc.gpsimd.affine_select(out=mask_add, in_=mask_add,
                                    compare_op=mybir.AluOpType.is_ge,
                                    fill=-1.0e6, base=0,
                                    pattern=[[-1, 128]], channel_multiplier=1)
            # multiplicative mask for [sk, sq] diag tiles: 1 if sk<=sq else 0
            mask_mulT = cp.tile([128, 128], BF)
            nc.vector.memset(mask_mulT, 1.0)
            nc.gpsimd.affine_select(out=mask_mulT, in_=mask_mulT,
                                    compare_op=mybir.AluOpType.is_ge,
                                    fill=0.0, base=0,
                                    pattern=[[1, 128]], channel_multiplier=-1)
            bq_sb = cp.tile([1, GH * HD], BF)
            nc.sync.dma_start(out=bq_sb, in_=bq[:, :])
            bk_sb = cp.tile([1, HD], BF)
            nc.sync.dma_start(out=bk_sb, in_=bk[:, :])
            bv_sb = cp.tile([1, HD], BF)
            nc.sync.dma_start(out=bv_sb, in_=bv[:, :])

            qT = qp.tile([128, GH, S], BF)      # [d, head, sq]
            kT = qp.tile([128, S], BF)          # [d, sk]
            v_bf = qp.tile([128, NT, HD], BF)   # [sk%128, sk//128, dv]
            wo_sb = wop.tile([128, GH, D], BF)
            nc.sync.dma_start(out=wo_sb,
                              in_=wo.ap().rearrange("(h p) n -> p h n", p=128))

            # ---- Phase 0: QKV projection + RoPE + transposes ----
            with (
                tc.tile_pool(name="wp", bufs=1) as wp,
                tc.tile_pool(name="xp", bufs=1) as xp,
                tc.tile_pool(name="rp", bufs=3) as rp,
                tc.tile_pool(name="pp0", bufs=2, space="PSUM") as pp0,
                tc.tile_pool(name="pt0", bufs=2, space="PSUM") as pt0,
            ):
                wq_sb = wp.tile([128, 16, GH * HD], BF)
                nc.sync.dma_start(out=wq_sb,
                                  in_=wq.ap().rearrange("(k p) n -> p k n", p=128))
                wk_sb = wp.tile([128, 16, HD], BF)
                nc.sync.dma_start(out=wk_sb,
                                  in_=wk.ap().rearrange("(k p) n -> p k n", p=128))
                wv_sb = wp.tile([128, 16, HD], BF)
                nc.sync.dma_start(out=wv_sb,
                                  in_=wv.ap().rearrange("(k p) n -> p k n", p=128))
                xT_sb = xp.tile([128, 16, S], BF)
                nc.sync.dma_start(out=xT_sb,
                                  in_=xT.ap().rearrange("(k p) s -> p k s", p=128))

                for i in range(NT):
                    r0, r1 = i * 128, i * 128 + 128
                    q_ps = pp0.tile([128, GH * HD], F32, tag="q_ps")
                    k_ps = pp0.tile([128, HD], F32, tag="k_ps", bufs=1)
                    v_ps = pp0.tile([128, HD], F32, tag="v_ps", bufs=1)
                    for k in range(16):
                        x_t = xT_sb[:, k, r0:r1]
                        nc.tensor.matmul(q_ps, x_t, wq_sb[:, k, :],
                                         start=(k == 0), stop=False)
                        nc.tensor.matmul(k_ps, x_t, wk_sb[:, k, :],
                                         start=(k == 0), stop=False)
                        nc.tensor.matmul(v_ps, x_t, wv_sb[:, k, :],
                                         start=(k == 0), stop=False)
                    nc.tensor.matmul(q_ps, ones1_bf, bq_sb, start=False, stop=True)
                    nc.tensor.matmul(k_ps, ones1_bf, bk_sb, start=False, stop=True)
                    nc.tensor.matmul(v_ps, ones1_bf, bv_sb, start=False, stop=True)
                    nc.vector.tensor_copy(v_bf[:, i, :], v_ps)

                    q_f = rp.tile([128, GH * HD], F32, tag="q_f")
                    nc.vector.tensor_copy(q_f, q_ps)
                    k_f = rp.tile([128, HD], F32, tag="k_f")
                    nc.vector.tensor_copy(k_f, k_ps)

                    cq4 = rp.tile([128, GH, 64], F32, tag="cq4")
                    nc.sync.dma_start(out=cq4, in_=cosq[r0:r1, :])
                    sq4 = rp.tile([128, GH, 64], F32, tag="sq4")
                    nc.sync.dma_start(out=sq4, in_=sinq[r0:r1, :])
                    ck = rp.tile([128, 64], F32, tag="ck")
                    nc.sync.dma_start(out=ck, in_=cosk[r0:r1, :])
                    sk_t = rp.tile([128, 64], F32, tag="sk_t")
                    nc.sync.dma_start(out=sk_t, in_=sink[r0:r1, :])

                    # RoPE on q (de-interleaved: per head [r(64) | i(64)])
                    q3 = q_f.rearrange("p (h x) -> p h x", h=GH)
                    qr, qi = q3[:, :, 0:64], q3[:, :, 64:128]
                    qrot = rp.tile([128, GH * HD], F32, tag="qrot")
                    o3 = qrot.rearrange("p (h x) -> p h x", h=GH)
                    orr, oi = o3[:, :, 0:64], o3[:, :, 64:128]
                    tmp = rp.tile([128, GH, 64], F32, tag="tmp")
                    nc.vector.tensor_mul(tmp, qi, sq4)
                    nc.vector.tensor_mul(orr, qr, cq4)
                    nc.vector.tensor_sub(orr, orr, tmp)
                    nc.vector.tensor_mul(tmp, qi, cq4)
                    nc.vector.tensor_mul(oi, qr, sq4)
                    nc.vector.tensor_add(oi, oi, tmp)
                    # RoPE on k
                    kr, ki = k_f[:, 0:64], k_f[:, 64:128]
                    krot = rp.tile([128, HD], F32, tag="krot")
                    krr, kio = krot[:, 0:64], krot[:, 64:128]
                    tmpk = rp.tile([128, 64], F32, tag="tmpk")
                    nc.vector.tensor_mul(tmpk, ki, sk_t)
                    nc.vector.tensor_mul(krr, kr, ck)
                    nc.vector.tensor_sub(krr, krr, tmpk)
                    nc.vector.tensor_mul(tmpk, ki, ck)
                    nc.vector.tensor_mul(kio, kr, sk_t)
                    nc.vector.tensor_add(kio, kio, tmpk)

                    # transposes -> qT / kT (cast to bf16 on copy-out)
                    for hh in range(GH):
                        tr_ps = pt0.tile([128, 128], F32, tag="tr")
                        nc.tensor.transpose(tr_ps, qrot[:, hh * HD:(hh + 1) * HD], ident)
                        nc.vector.tensor_copy(qT[:, hh, r0:r1], tr_ps)
                    tr_ps = pt0.tile([128, 128], F32, tag="tr")
                    nc.tensor.transpose(tr_ps, krot, ident)
                    nc.vector.tensor_copy(kT[:, r0:r1], tr_ps)

            # ---- Attention per head ----
            with (
                tc.tile_pool(name="etp", bufs=1) as etp,
                tc.tile_pool(name="otp", bufs=1) as otp,
                tc.tile_pool(name="sbm", bufs=2) as sbm,
                tc.tile_pool(name="sbB", bufs=2) as sbB,
                tc.tile_pool(name="ppA", bufs=2, space="PSUM") as ppA,
                tc.tile_pool(name="ppB", bufs=2, space="PSUM") as ppB,
                tc.tile_pool(name="ppC", bufs=2, space="PSUM") as ppC,
                tc.tile_pool(name="ppM", bufs=1, space="PSUM") as ppM,
            ):
                oT = otp.tile([128, GH, S], BF)   # [dv, head, sq]
                for hh in range(GH):
                    eT = etp.tile([128, NT, S], BF, tag="eT")  # [sk%128, sk//128, sq]
                    # Phase A: transposed scores -> exp -> eT
                    for j in range(NT):
                        for c in range(j // 4, NC):
                            sT_ps = ppA.tile([128, 512], F32, tag="sT")
                            nc.tensor.matmul(sT_ps, kT[:, j * 128:(j + 1) * 128],
                                             qT[:, hh, c * 512:(c + 1) * 512],
                                             start=True, stop=True)
                            nc.scalar.activation(eT[:, j, c * 512:(c + 1) * 512],
                                                 sT_ps, AF.Exp, scale=SCALE)
                        off = 128 * (j % 4)
                        if off:
                            nc.vector.memset(eT[:, j, (j // 4) * 512:(j // 4) * 512 + off], 0.0)
                        nc.vector.tensor_mul(eT[:, j, j * 128:(j + 1) * 128],
                                             eT[:, j, j * 128:(j + 1) * 128], mask_mulT)

                    # Phase A2 + C: rowsums, broadcast recip, AV, normalized oT
                    recip_row = sbm.tile([1, S], F32, tag="recip_row", bufs=1)
                    for c in range(NC):
                        cs = slice(c * 512, (c + 1) * 512)
                        r_ps = ppM.tile([1, 512], F32, tag="r_ps")
                        for j in range(4 * c + 4):
                            nc.tensor.matmul(r_ps, ones_bf, eT[:, j, cs],
                                             start=(j == 0), stop=(j == 4 * c + 3))
                        nc.vector.reciprocal(recip_row[0:1, cs], r_ps)
                        bc_ps = ppM.tile([128, 512], F32, tag="bc_ps")
                        nc.tensor.matmul(bc_ps, ones_row_f, recip_row[0:1, cs],
                                         start=True, stop=True)
                        bc_sb = sbm.tile([128, 512], F32, tag="bc_sb")
                        nc.vector.tensor_copy(bc_sb, bc_ps)
                        oT_ps = ppC.tile([128, 512], F32, tag="oT")
                        for j in range(4 * c + 4):
                            nc.tensor.matmul(oT_ps, v_bf[:, j, :], eT[:, j, cs],
                                             start=(j == 0), stop=(j == 4 * c + 3))
                        nc.vector.tensor_mul(oT[:, hh, cs], oT_ps, bc_sb)

                    # Phase B: attn rows [sq, sk], fp32, normalized, DMA out
                    for i in range(NT):
                        W = 128 * (i + 1)
                        racc = sbB.tile([128, NC], F32, tag="racc")
                        e_f = sbB.tile([128, S], F32, tag="e_f")
                        for c in range(i // 4 + 1):
                            w = 512 if c < i // 4 else 128 * (i % 4) + 128
                            s_ps = ppB.tile([128, 512], F32, tag="s_ps")
                            nc.tensor.matmul(s_ps[:, :w], qT[:, hh, i * 128:(i + 1) * 128],
                                             kT[:, c * 512:c * 512 + w],
                                             start=True, stop=True)
                            if c == i // 4:
                                nc.vector.tensor_add(s_ps[:, w - 128:w],
                                                     s_ps[:, w - 128:w], mask_add)
                            nc.scalar.activation(e_f[:, c * 512:c * 512 + w],
                                                 s_ps[:, :w], AF.Exp, scale=SCALE,
                                                 accum_out=racc[:, c:c + 1])
                        r1t = sbB.tile([128, 1], F32, tag="r1t")
                        nc.vector.tensor_reduce(r1t, racc[:, 0:i // 4 + 1],
                                                axis=mybir.AxisListType.X,
                                                op=mybir.AluOpType.add)
                        rr = sbB.tile([128, 1], F32, tag="rr")
                        nc.vector.reciprocal(rr, r1t)
                        attn_row = sbB.tile([128, S], F32, tag="attn_row")
                        nc.vector.tensor_scalar_mul(attn_row[:, :W], e_f[:, :W], rr)
                        nc.sync.dma_start(out=attn_o[hh][i * 128:(i + 1) * 128, 0:W],
                                          in_=attn_row[:, :W])

            # ---- Phase D: out = sum_h oT_h^T @ wo_h (partial over this group) ----
            with (
                tc.tile_pool(name="sbD", bufs=3) as sbD,
                tc.tile_pool(name="ppD", bufs=2, space="PSUM") as ppD,
            ):
                for i in range(NT):
                    out_row = sbD.tile([128, D], F32, tag="out_row")
                    for m in range(4):
                        o_ps = ppD.tile([128, 512], F32, tag="o_ps")
                        for hh in range(GH):
                            nc.tensor.matmul(o_ps, oT[:, hh, i * 128:(i + 1) * 128],
                                             wo_sb[:, hh, m * 512:(m + 1) * 512],
                                             start=(hh == 0), stop=(hh == GH - 1))
                        nc.vector.tensor_copy(out_row[:, m * 512:(m + 1) * 512], o_ps)
                    nc.sync.dma_start(out=outp[i * 128:(i + 1) * 128, :], in_=out_row)

    nc.finalize()
    return nc


_PERM = np.concatenate([np.arange(0, HD, 2), np.arange(1, HD, 2)])


def _prep_inputs(x, freqs_cos, freqs_sin, wq, bq, wk, bk, wv, bv, wo):
    bf = ml_dtypes.bfloat16
    idx_g = np.concatenate([hh * HD + _PERM for hh in range(GH)])
    cosq = np.tile(freqs_cos, (1, GH)).astype(np.float32)
    sinq = np.tile(freqs_sin, (1, GH)).astype(np.float32)
    cosk = np.ascontiguousarray(freqs_cos, dtype=np.float32)
    sink = np.ascontiguousarray(freqs_sin, dtype=np.float32)
    in_maps = []
    for core in range(8):
        b, g = core // HKV, core % HKV
        q0 = g * GH * HD
        in_maps.append({
            "xT": np.ascontiguousarray(x[b].T).astype(bf),
            "wq": np.ascontiguousarray(wq[:, q0:q0 + GH * HD][:, idx_g]).astype(bf),
            "wk": np.ascontiguousarray(wk[:, g * HD:(g + 1) * HD][:, _PERM]).astype(bf),
            "wv": np.ascontiguousarray(wv[:, g * HD:(g + 1) * HD]).astype(bf),
            "wo": np.ascontiguousarray(wo[q0:q0 + GH * HD, :]).astype(bf),
            "bq": bq[q0:q0 + GH * HD][idx_g].reshape(1, -1).astype(bf),
            "bk": bk[g * HD:(g + 1) * HD][_PERM].reshape(1, -1).astype(bf),
            "bv": bv[g * HD:(g + 1) * HD].reshape(1, -1).astype(bf),
            "cosq": cosq, "sinq": sinq, "cosk": cosk, "sink": sink,
        })
    return in_maps


def kernel(x, freqs_cos, freqs_sin, wq, bq, wk, bk, wv, bv, wo):
    global _CACHED_NC
    x = np.asarray(x, dtype=np.float32)
    in_maps = _prep_inputs(np.asarray(x), np.asarray(freqs_cos), np.asarray(freqs_sin),
                           np.asarray(wq), np.asarray(bq), np.asarray(wk),
                           np.asarray(bk), np.asarray(wv), np.asarray(bv),
                           np.asarray(wo))
    if _CACHED_NC is None:
        _CACHED_NC = build_nc()
    res = run_bass_kernel_spmd(_CACHED_NC, in_maps, core_ids=list(range(8)))

    out = np.zeros((B, S, D), dtype=np.float32)
    attn = np.empty((B, HQ, S, S), dtype=np.float32)
    for core in range(8):
        b, g = core // HKV, core % HKV
        r = res.results[core]
        out[b] += r["outp"]
        for hh in range(GH):
            attn[b, g * GH + hh] = r[f"attn{hh}"]
    return out, attn
